# revision 12
# baseline (speedup 1.0000x reference)
"""Trainium2 Bass kernel for nn_PhongBase.

Math (per row n of inputs[N, 3, 3]):
    light  = inputs[n, 0, :]
    normal = inputs[n, 1, :]
    ndotl  = max(dot(light, normal), 0)
    out[n, j] = ks[j]/pi + kd[j]/pi * ndotl          (j = 0..2)

(The view vector inputs[n, 2, :] is unused; specular() == 1.0**alpha == 1.)

Strategy: pure data parallel over 8 NeuronCores. Each core gets N/8 rows as a
contiguous block, streams [128, W, 9] f32 tiles HBM->SBUF with fully
contiguous DMA, computes the dot product with strided DVE ops, applies the
per-channel affine (scale = kd/pi, bias = ks/pi, folded in as compile-time
immediates) on the scalar engine, and streams [128, W, 3] tiles back.
"""

import numpy as np

import concourse.bacc as bacc
import concourse.bass as bass
import concourse.mybir as mybir
from concourse.bass_utils import run_bass_kernel_spmd
from concourse.tile import TileContext

N_CORES = 8
N_ROWS = 8388608                  # full batch
P = 128                           # SBUF partitions
W = 1024                          # rows per partition per tile
T = N_ROWS // (N_CORES * P * W)   # tiles per core (= 8)

F32 = mybir.dt.float32
INV_PI = 1.0 / np.pi


def build_program(scale3, bias3, tiles=T, w=W, strided_read=True):
    """Bass program for one core: x[tiles,P,w,9] -> y[tiles,P,w,3].

    out[..., j] = scale3[j] * relu(sum(x[..., 0:3] * x[..., 3:6], -1)) + bias3[j]

    strided_read: load only the 6 needed floats of each 9-float row
    (skips the unused view vector -> 33% less read traffic).
    """
    # Bacc (not plain Bass): its finalization passes split multi-wait
    # instructions via event semaphores — TRN2 allows 1 sync wait per inst.
    nc = bacc.Bacc(None)
    x = nc.dram_tensor("x", [tiles, P, w, 9], F32, kind="ExternalInput")
    y = nc.dram_tensor("y", [tiles, P, w, 3], F32, kind="ExternalOutput")
    c = 6 if strided_read else 9

    with TileContext(nc) as tc:
        with (
            tc.tile_pool(name="in_pool", bufs=3) as in_pool,
            tc.tile_pool(name="out_pool", bufs=3) as out_pool,
            tc.tile_pool(name="prod_pool", bufs=3) as prod_pool,
            tc.tile_pool(name="dot_pool", bufs=3) as dot_pool,
        ):
            for t in range(tiles):
                itile = in_pool.tile([P, w, c], F32)
                otile = out_pool.tile([P, w, 3], F32)
                prod = prod_pool.tile([P, w, 3], F32)
                dot = dot_pool.tile([P, w], F32)
                relu = dot_pool.tile([P, w], F32, tag="relu")

                if strided_read:
                    # strided source AP: [P]x[w] row dims merge in lowering,
                    # and the merged count must fit a 16-bit ISA field --
                    # split along w so each DMA is [P, w//4, 6] (32768 <= 65535)
                    wq = w // 4
                    for i in range(4):
                        nc.sync.dma_start(
                            out=itile[:, i * wq : (i + 1) * wq, :],
                            in_=x[t][:, i * wq : (i + 1) * wq, 0:6],
                        )
                else:
                    nc.sync.dma_start(out=itile[:], in_=x[t])

                light = itile[:, :, 0:3]
                normal = itile[:, :, 3:6]
                # dot product on DVE: one 3w-elem mul + two w-elem adds
                nc.vector.tensor_mul(out=prod[:], in0=light, in1=normal)
                nc.vector.tensor_add(
                    out=dot[:], in0=prod[:, :, 0], in1=prod[:, :, 1]
                )
                nc.vector.tensor_add(out=dot[:], in0=dot[:], in1=prod[:, :, 2])
                # relu + per-channel affine on the scalar engine (ACT),
                # balancing engine load; DVE would otherwise co-bottleneck
                # with DMA. Copy computes out = in*scale + bias.
                nc.scalar.activation(
                    out=relu[:],
                    in_=dot[:],
                    func=mybir.ActivationFunctionType.Relu,
                )
                for j in range(3):
                    nc.scalar.activation(
                        out=otile[:, :, j],
                        in_=relu[:],
                        func=mybir.ActivationFunctionType.Copy,
                        bias=float(bias3[j]),
                        scale=float(scale3[j]),
                    )

                # store on the scalar-engine HWDGE ring (qScalarDynamicHW):
                # HWDGE DMAs are FIFO per issuing engine, so stores on the
                # sync ring would head-of-line block later loads.
                nc.scalar.dma_start(out=y[t], in_=otile[:])
    return nc


def run_sharded(x_np, scale3, bias3, **spmd_kwargs):
    """Shard x_np [N_ROWS, 3, 3] over 8 cores, run, gather [N_ROWS, 3]."""
    rows_per_core = N_ROWS // N_CORES
    x5 = np.ascontiguousarray(x_np, dtype=np.float32).reshape(
        N_CORES, T, P, W, 9
    )
    nc = build_program(scale3, bias3)
    nc.finalize()  # run Bacc's compile pipeline (wait splitting, reg alloc)
    in_maps = [{"x": x5[c]} for c in range(N_CORES)]
    rr = run_bass_kernel_spmd(nc, in_maps, list(range(N_CORES)), **spmd_kwargs)
    out = np.empty((N_CORES, rows_per_core, 3), dtype=np.float32)
    for c in range(N_CORES):
        out[c] = np.asarray(rr.results[c]["y"]).reshape(rows_per_core, 3)
    return out.reshape(N_ROWS, 3), rr


def kernel(inputs, kd, ks, alpha):
    inputs = np.asarray(inputs, dtype=np.float32)
    kd = np.asarray(kd, dtype=np.float32)
    ks = np.asarray(ks, dtype=np.float32)
    alpha = np.asarray(alpha, dtype=np.float32)

    inv_pi = np.float32(INV_PI)
    spec = np.float32(1.0) ** alpha          # specular() of the base class
    scale3 = (kd * inv_pi).astype(np.float32)          # per-channel scale
    bias3 = (ks * inv_pi * spec).astype(np.float32)    # per-channel bias

    out, _ = run_sharded(inputs, scale3, bias3)
    return out


# revision 15
# speedup vs baseline: 4.4489x; 4.4489x over previous
"""Trainium2 Bass kernel for nn_PhongBase.

Math (per row n of inputs[N, 3, 3]):
    light  = inputs[n, 0, :]
    normal = inputs[n, 1, :]
    ndotl  = max(dot(light, normal), 0)
    out[n, j] = ks[j]/pi + kd[j]/pi * ndotl          (j = 0..2)

(The view vector inputs[n, 2, :] is unused; specular() == 1.0**alpha == 1.)

Strategy: pure data parallel over 8 NeuronCores; each core streams its
contiguous N/8-row block through SBUF. Per tile of (128 partitions x w rows):
  - contiguous HBM->SBUF load of [128, w, 9] f32 on the sync HWDGE ring
  - dot product on DVE (one 3w mul + two w adds, strided APs)
  - relu + per-channel affine (Copy: out = in*scale + bias with the kd/pi,
    ks/pi constants folded in as immediates) on the scalar engine
  - contiguous [128, w, 3] store on the scalar HWDGE ring (separate ring so
    stores never head-of-line block loads: HWDGE is FIFO per issuing engine)
The tile schedule tapers (7x1024 + 4x256 rows/partition) so the pipeline
drain after the last load is short; the kernel is HBM-bandwidth-bound at
~358 GB/s per core.
"""

import numpy as np

import concourse.bacc as bacc
import concourse.mybir as mybir
from concourse.bass_utils import run_bass_kernel_spmd
from concourse.tile import TileContext

N_CORES = 8
N_ROWS = 8388608                  # full batch
P = 128                           # SBUF partitions
RPP = N_ROWS // (N_CORES * P)     # rows per partition per core (= 8192)
SCHEDULE = [512] * 14 + [256] * 4  # sums to RPP

F32 = mybir.dt.float32
INV_PI = 1.0 / np.pi


def build_program(scale3, bias3, schedule=SCHEDULE):
    """Bass program for one core: x[rows,9] -> y[rows,3], rows = P*sum(schedule).

    out[r, j] = scale3[j] * relu(sum(x[r, 0:3] * x[r, 3:6], -1)) + bias3[j]
    """
    rows = P * sum(schedule)
    # Bacc (not plain Bass): its finalization passes split multi-wait
    # instructions via event semaphores — TRN2 allows 1 sync wait per inst.
    nc = bacc.Bacc(None)
    x = nc.dram_tensor("x", [rows, 9], F32, kind="ExternalInput")
    y = nc.dram_tensor("y", [rows, 3], F32, kind="ExternalOutput")

    with TileContext(nc) as tc:
        with (
            tc.tile_pool(name="in_pool", bufs=4) as in_pool,
            tc.tile_pool(name="out_pool", bufs=4) as out_pool,
            tc.tile_pool(name="dot_pool", bufs=4) as dot_pool,
        ):
            off = 0
            for w in schedule:
                itile = in_pool.tile([P, w, 9], F32, tag="itile")
                otile = out_pool.tile([P, w, 3], F32, tag="otile")
                dot = dot_pool.tile([P, w], F32, tag="dot")

                src = x[off : off + P * w].rearrange("(p w) c -> p w c", p=P)
                dst = y[off : off + P * w].rearrange("(p w) c -> p w c", p=P)
                off += P * w

                nc.sync.dma_start(out=itile[:], in_=src)

                light = itile[:, :, 0:3]
                normal = itile[:, :, 3:6]
                # dot product on DVE: one 3w-elem mul (otile as scratch)
                # + two w-elem adds
                nc.vector.tensor_mul(out=otile[:], in0=light, in1=normal)
                nc.vector.tensor_add(
                    out=dot[:], in0=otile[:, :, 0], in1=otile[:, :, 1]
                )
                nc.vector.tensor_add(out=dot[:], in0=dot[:], in1=otile[:, :, 2])
                # relu + per-channel affine on the scalar engine (ACT),
                # balancing engine load; DVE would otherwise co-bottleneck
                # with DMA. Copy computes out = in*scale + bias.
                nc.scalar.activation(
                    out=dot[:],
                    in_=dot[:],
                    func=mybir.ActivationFunctionType.Relu,
                )
                for j in range(3):
                    nc.scalar.activation(
                        out=otile[:, :, j],
                        in_=dot[:],
                        func=mybir.ActivationFunctionType.Copy,
                        bias=float(bias3[j]),
                        scale=float(scale3[j]),
                    )

                nc.scalar.dma_start(out=dst, in_=otile[:])
    return nc


def run_sharded(x_np, scale3, bias3, **spmd_kwargs):
    """Shard x_np [N_ROWS, 3, 3] over 8 cores, run, gather [N_ROWS, 3]."""
    rows_per_core = N_ROWS // N_CORES
    x3 = np.ascontiguousarray(x_np, dtype=np.float32).reshape(
        N_CORES, rows_per_core, 9
    )
    nc = build_program(scale3, bias3)
    nc.finalize()  # run Bacc's compile pipeline (wait splitting, reg alloc)
    in_maps = [{"x": x3[c]} for c in range(N_CORES)]
    rr = run_bass_kernel_spmd(nc, in_maps, list(range(N_CORES)), **spmd_kwargs)
    out = np.empty((N_CORES, rows_per_core, 3), dtype=np.float32)
    for c in range(N_CORES):
        out[c] = np.asarray(rr.results[c]["y"])
    return out.reshape(N_ROWS, 3), rr


def kernel(inputs, kd, ks, alpha):
    inputs = np.asarray(inputs, dtype=np.float32)
    kd = np.asarray(kd, dtype=np.float32)
    ks = np.asarray(ks, dtype=np.float32)
    alpha = np.asarray(alpha, dtype=np.float32)

    inv_pi = np.float32(INV_PI)
    spec = np.float32(1.0) ** alpha          # specular() of the base class
    scale3 = (kd * inv_pi).astype(np.float32)          # per-channel scale
    bias3 = (ks * inv_pi * spec).astype(np.float32)    # per-channel bias

    out, _ = run_sharded(inputs, scale3, bias3)
    return out


# revision 16
# speedup vs baseline: 5.2468x; 1.1793x over previous
"""Trainium2 Bass kernel for nn_PhongBase.

Math (per row n of inputs[N, 3, 3]):
    light  = inputs[n, 0, :]
    normal = inputs[n, 1, :]
    ndotl  = max(dot(light, normal), 0)
    out[n, j] = ks[j]/pi + kd[j]/pi * ndotl          (j = 0..2)

(The view vector inputs[n, 2, :] is unused; specular() == 1.0**alpha == 1.)

Strategy: pure data parallel over 8 NeuronCores; each core streams its
contiguous N/8-row block through SBUF. Per tile of (128 partitions x w rows):
  - contiguous HBM->SBUF load of [128, w, 9] f32 on the sync HWDGE ring
  - dot product on DVE (one 3w mul + two w adds, strided APs)
  - relu + per-channel affine (Copy: out = in*scale + bias with the kd/pi,
    ks/pi constants folded in as immediates) on the scalar engine
  - contiguous [128, w, 3] store on the scalar HWDGE ring (separate ring so
    stores never head-of-line block loads: HWDGE is FIFO per issuing engine)
The tile schedule tapers (7x1024 + 4x256 rows/partition) so the pipeline
drain after the last load is short; the kernel is HBM-bandwidth-bound at
~358 GB/s per core.
"""

import numpy as np

import concourse.bacc as bacc
import concourse.mybir as mybir
from concourse.bass_utils import run_bass_kernel_spmd
from concourse.tile import TileContext

N_CORES = 8
N_ROWS = 8388608                  # full batch
P = 128                           # SBUF partitions
RPP = N_ROWS // (N_CORES * P)     # rows per partition per core (= 8192)
SCHEDULE = [512] * 14 + [256] * 2 + [128] * 2 + [64] * 4  # sums to RPP

F32 = mybir.dt.float32
INV_PI = 1.0 / np.pi


def build_program(scale3, bias3, schedule=SCHEDULE):
    """Bass program for one core: x[rows,9] -> y[rows,3], rows = P*sum(schedule).

    out[r, j] = scale3[j] * relu(sum(x[r, 0:3] * x[r, 3:6], -1)) + bias3[j]
    """
    rows = P * sum(schedule)
    # Bacc (not plain Bass): its finalization passes split multi-wait
    # instructions via event semaphores — TRN2 allows 1 sync wait per inst.
    nc = bacc.Bacc(None)
    x = nc.dram_tensor("x", [rows, 9], F32, kind="ExternalInput")
    y = nc.dram_tensor("y", [rows, 3], F32, kind="ExternalOutput")

    with TileContext(nc) as tc:
        with (
            tc.tile_pool(name="in_pool", bufs=4) as in_pool,
            tc.tile_pool(name="out_pool", bufs=4) as out_pool,
            tc.tile_pool(name="dot_pool", bufs=4) as dot_pool,
        ):
            off = 0
            for w in schedule:
                itile = in_pool.tile([P, w, 9], F32, tag="itile")
                otile = out_pool.tile([P, w, 3], F32, tag="otile")
                dot = dot_pool.tile([P, w], F32, tag="dot")

                src = x[off : off + P * w].rearrange("(p w) c -> p w c", p=P)
                dst = y[off : off + P * w].rearrange("(p w) c -> p w c", p=P)
                off += P * w

                nc.sync.dma_start(out=itile[:], in_=src)

                light = itile[:, :, 0:3]
                normal = itile[:, :, 3:6]
                # dot product on DVE: one 3w-elem mul (otile as scratch)
                # + two w-elem adds
                nc.vector.tensor_mul(out=otile[:], in0=light, in1=normal)
                nc.vector.tensor_add(
                    out=dot[:], in0=otile[:, :, 0], in1=otile[:, :, 1]
                )
                nc.vector.tensor_add(out=dot[:], in0=dot[:], in1=otile[:, :, 2])
                # relu + per-channel affine on the scalar engine (ACT),
                # balancing engine load; DVE would otherwise co-bottleneck
                # with DMA. Copy computes out = in*scale + bias.
                nc.scalar.activation(
                    out=dot[:],
                    in_=dot[:],
                    func=mybir.ActivationFunctionType.Relu,
                )
                for j in range(3):
                    nc.scalar.activation(
                        out=otile[:, :, j],
                        in_=dot[:],
                        func=mybir.ActivationFunctionType.Copy,
                        bias=float(bias3[j]),
                        scale=float(scale3[j]),
                    )

                nc.scalar.dma_start(out=dst, in_=otile[:])
    return nc


def run_sharded(x_np, scale3, bias3, **spmd_kwargs):
    """Shard x_np [N_ROWS, 3, 3] over 8 cores, run, gather [N_ROWS, 3]."""
    rows_per_core = N_ROWS // N_CORES
    x3 = np.ascontiguousarray(x_np, dtype=np.float32).reshape(
        N_CORES, rows_per_core, 9
    )
    nc = build_program(scale3, bias3)
    nc.finalize()  # run Bacc's compile pipeline (wait splitting, reg alloc)
    in_maps = [{"x": x3[c]} for c in range(N_CORES)]
    rr = run_bass_kernel_spmd(nc, in_maps, list(range(N_CORES)), **spmd_kwargs)
    out = np.empty((N_CORES, rows_per_core, 3), dtype=np.float32)
    for c in range(N_CORES):
        out[c] = np.asarray(rr.results[c]["y"])
    return out.reshape(N_ROWS, 3), rr


def kernel(inputs, kd, ks, alpha):
    inputs = np.asarray(inputs, dtype=np.float32)
    kd = np.asarray(kd, dtype=np.float32)
    ks = np.asarray(ks, dtype=np.float32)
    alpha = np.asarray(alpha, dtype=np.float32)

    inv_pi = np.float32(INV_PI)
    spec = np.float32(1.0) ** alpha          # specular() of the base class
    scale3 = (kd * inv_pi).astype(np.float32)          # per-channel scale
    bias3 = (ks * inv_pi * spec).astype(np.float32)    # per-channel bias

    out, _ = run_sharded(inputs, scale3, bias3)
    return out
